# revision 32
# baseline (speedup 1.0000x reference)
"""Trainium2 Bass kernel: multi-head multi-resolution location-aware attention.

Math (per batch b, head h):
  K      = enc @ Wk[h].T                      (T, DK)   <- heavy matmul, bf16 on PE
  conv_h = depthwise 1-D conv of att_prev     (C, T)
  att    = conv_h.T @ Watt[h]                 (T, DK)
  Q      = Wq[h] @ dec_z + bq                 (DK,)
  e[t]   = g_w[h] . tanh(K[t] + att[t] + Q) + g_b[h]
  w      = softmax(scale * mask(e))           (T,)
  c_h    = Wv[h] @ (enc.T @ w)                <- V-projection eliminated algebraically
  c      = concat_h(c_h) @ Wo.T

Sharding: data-parallel over batch B=16 across 8 cores (2 per core), no collectives.
Each PSUM K-tile is [128t, 512k]; accumulation = 8 e-chunks + att_feat + Q (ones-row),
with all 4 heads sharing each stationary encT slice across parallel PSUM banks.
tanh on ScalarE (bf16 out), g*tanh -> sum_k as DVE mul+reduce (bf16, 2x mode),
softmax on-chip (no max-subtraction needed: |logits| < 1), context path in f32:
m = enc.T @ w on PE with w stationary, then Wv, then Wo. The im2col/encT loads are
issued two chunks ahead of their matmuls; big weight DMAs are split in halves so
transfers ride two HWDGE rings in parallel.
"""

import numpy as np
import ml_dtypes
from contextlib import ExitStack

import concourse.bass as bass
import concourse.bacc as bacc
import concourse.mybir as mybir
import concourse.tile as tile

F32 = mybir.dt.float32
BF16 = mybir.dt.bfloat16

# problem constants (hardcoded per contract)
B, T, E, DU = 16, 2048, 1024, 1024
H, DK, DV, C = 4, 512, 512, 128
HD = H * DV                      # 2048
NCORES = 8
BL = B // NCORES                 # 2 batches per core
P = 128
AF = [25, 50, 75, 100]           # per-head conv half-widths
W = [2 * a + 1 for a in AF]      # 51, 101, 151, 201
# per-head (offset, count) chunks of the conv-tap dimension
CHUNKS = [[(0, w)] if w <= P else [(0, P), (P, w - P)] for w in W]
XOFF = 128                       # left zero-pad of the conv input
TP = T + 2 * XOFF                # padded conv-input length
SCALING = 1.0 / float(np.sqrt(DK))
NEG = -1.0e30

NT = T // P                      # 16 t-tiles of 128
NTC = T // 512                   # 4 t-chunks of 512
NE = E // P                      # 8 e-chunks of 128
DUP = DU + P                     # dec_z padded with a constant-1 row (folds bq in)
NDC = DUP // P                   # 9 contraction chunks for the Q projection


def _emit(tc, ap, stages=99):
    nc = tc.nc
    ctx = ExitStack()

    sing = ctx.enter_context(tc.tile_pool(name="sing", bufs=1))
    psum = ctx.enter_context(tc.tile_pool(name="psum", bufs=8, space="PSUM"))
    wqp = ctx.enter_context(tc.tile_pool(name="wqp", bufs=1))
    encTp = ctx.enter_context(tc.tile_pool(name="encTp", bufs=3))
    xtp = ctx.enter_context(tc.tile_pool(name="xtp", bufs=12))
    convp = ctx.enter_context(tc.tile_pool(name="convp", bufs=8))
    tanhp = ctx.enter_context(tc.tile_pool(name="tanhp", bufs=5))
    scrp = ctx.enter_context(tc.tile_pool(name="scrp", bufs=3))
    softp = ctx.enter_context(tc.tile_pool(name="softp", bufs=2))
    wstp = ctx.enter_context(tc.tile_pool(name="wstp", bufs=2))
    encNp = ctx.enter_context(tc.tile_pool(name="encNp", bufs=3))
    mp = ctx.enter_context(tc.tile_pool(name="mp", bufs=2))
    wvp = ctx.enter_context(tc.tile_pool(name="wvp", bufs=1))
    wop = ctx.enter_context(tc.tile_pool(name="wop", bufs=2))
    cp = ctx.enter_context(tc.tile_pool(name="cp", bufs=2))

    with ctx:
        # ---- persistent weights -------------------------------------------------
        sb_wkT = []
        sb_wattT = []
        sb_gw = []
        sb_cw = []
        for h in range(H):
            wk = sing.tile([P, NE, DK], BF16, tag=f"wkT{h}", name=f"wkT{h}")
            nc.sync.dma_start(out=wk, in_=ap["wkT"][h].rearrange("(c p) k -> p c k", p=P))
            sb_wkT.append(wk)
            wa = sing.tile([C, DK], BF16, tag=f"wattT{h}", name=f"wattT{h}")
            nc.sync.dma_start(out=wa, in_=ap["wattT"][h])
            sb_wattT.append(wa)
            g_row = ap["gw"][h:h + 1, :]
            g_bc = bass.AP(tensor=g_row.tensor, offset=g_row.offset, ap=[[0, P], g_row.ap[1]])
            gt = sing.tile([P, DK], BF16, tag=f"gw{h}", name=f"gw{h}")
            nc.sync.dma_start(out=gt, in_=g_bc)
            sb_gw.append(gt)
            cws = []
            for j, (dofs, dcnt) in enumerate(CHUNKS[h]):
                cw = sing.tile([dcnt, C], BF16, tag=f"cw{h}_{j}", name=f"cw{h}_{j}")
                nc.sync.dma_start(out=cw, in_=ap[f"cw{h}"][dofs:dofs + dcnt, :])
                cws.append(cw)
            sb_cw.append(cws)

        sb_I = sing.tile([P, P], F32, tag="ident", name="identt")
        nc.sync.dma_start(out=sb_I, in_=ap["ident"])
        sb_dzT = sing.tile([P, NDC, BL], BF16, tag="dzT", name="dzTt")
        nc.sync.dma_start(out=sb_dzT, in_=ap["dzT"].rearrange("(c p) b -> p c b", p=P))
        sb_ones_b = sing.tile([1, P], BF16, tag="ones_b", name="ones_b")
        nc.vector.memset(sb_ones_b, 1.0)
        sb_ones_f = sing.tile([P, P], F32, tag="ones_f", name="ones_f")
        nc.vector.memset(sb_ones_f, 1.0)

        q_sb = [[sing.tile([1, DK], BF16, tag=f"q{h}_{b}", name=f"q{h}_{b}")
                 for b in range(BL)] for h in range(H)]
        m2 = sing.tile([P, NE, H, BL], F32, tag="m2", name="m2_sb")
        cc = sing.tile([P, HD // P, BL], F32, tag="cc", name="cc_sb")

        # ---- Q projection: q[h][b] = Wq[h] @ dec_z[b] + bq[h] (bq folded in) ---
        for h in range(H):
            wq_t = wqp.tile([P, NDC, DK], BF16, tag="wq", name=f"wq{h}")
            wq_v = ap["wqT"][h].rearrange("(c p) k -> p c k", p=P)
            nc.sync.dma_start(out=wq_t[:, 0:5, :], in_=wq_v[:, 0:5, :])
            nc.sync.dma_start(out=wq_t[:, 5:NDC, :], in_=wq_v[:, 5:NDC, :])
            pqs = [psum.tile([1, DK], F32, tag="ps", name=f"pq{h}_{b}")
                   for b in range(BL)]
            for dc in range(NDC):
                for b in range(BL):
                    nc.tensor.matmul(pqs[b], sb_dzT[:, dc, b:b + 1], wq_t[:, dc, :],
                                     start=(dc == 0), stop=(dc == NDC - 1))
            for b in range(BL):
                nc.vector.tensor_copy(q_sb[h][b], pqs[b])

        # ---- main loop: K-groups for BOTH batches first (keeps PE dense; the
        # ---- softmax/context phases below overlap with the other batch) --------
        if stages < 2:
            return
        def emit_dmas(b, tcc):
            """Issue the encT + im2col DMAs for (b, tcc) — two chunks ahead of
            the K-groups so transfers land long before the PE needs them."""
            encT_b = ap["encT"][b].rearrange("(c p) t -> p c t", p=P)
            et = encTp.tile([P, NE, 512], BF16, tag="encT", name=f"encT{b}_{tcc}")
            nc.sync.dma_start(out=et[:, 0:4, :],
                              in_=encT_b[:, 0:4, tcc * 512:(tcc + 1) * 512])
            nc.sync.dma_start(out=et[:, 4:NE, :],
                              in_=encT_b[:, 4:NE, tcc * 512:(tcc + 1) * 512])
            xts = []
            for h in range(H):
                for j, (dofs, dcnt) in enumerate(CHUNKS[h]):
                    xt = xtp.tile([P, 512], BF16, tag="xt", name=f"xt{b}_{tcc}_{h}_{j}")
                    base = b * TP + XOFF - AF[h] + tcc * 512 + dofs
                    src = bass.AP(tensor=ap["xpad"].tensor, offset=base,
                                  ap=[[1, dcnt], [1, 512]])
                    nc.sync.dma_start(out=xt[:dcnt, :], in_=src)
                    xts.append(xt)
            return et, xts

        def emit_conv_mms(b, tcc, xts):
            """Conv matmuls + PSUM->SBUF copies; emitted after the previous
            chunk's K-groups (PE reaches them with the DMA data already in)."""
            convs = []
            k = 0
            for h in range(H):
                pc = psum.tile([C, 512], F32, tag="ps", name=f"pconv{b}_{tcc}_{h}")
                nch = len(CHUNKS[h])
                for j, (dofs, dcnt) in enumerate(CHUNKS[h]):
                    nc.tensor.matmul(pc, sb_cw[h][j], xts[k][:dcnt, :],
                                     start=(j == 0), stop=(j == nch - 1))
                    k += 1
                cv = convp.tile([C, 512], BF16, tag="conv", name=f"conv{b}_{tcc}_{h}")
                nc.vector.tensor_copy(cv, pc)
                convs.append(cv)
            return convs

        e_sbs = [softp.tile([P, NT, H], F32, tag="e", name=f"e{b}")
                 for b in range(BL)]
        bt = [(b, tcc) for b in range(BL) for tcc in range(NTC)]
        dmas = {0: emit_dmas(*bt[0])}
        convd = {0: emit_conv_mms(*bt[0], dmas[0][1])}
        dmas[1] = emit_dmas(*bt[1])
        for idx, (b, tcc) in enumerate(bt):
            e_sb = e_sbs[b]
            et, _ = dmas.pop(idx)
            convs = convd.pop(idx)
            if idx + 2 < len(bt):
                dmas[idx + 2] = emit_dmas(*bt[idx + 2])
            if idx + 1 < len(bt):
                convd[idx + 1] = emit_conv_mms(*bt[idx + 1], dmas[idx + 1][1])
            if True:
                for tt in range(4):
                    ti = tcc * 4 + tt
                    # all 4 heads accumulate in parallel PSUM banks so each
                    # stationary encT slice is loaded once and reused 4x
                    pks = [psum.tile([P, DK], F32, tag="ps", name=f"pk{b}_{ti}_{h}")
                           for h in range(H)]
                    for ec in range(NE):
                        ets = et[:, ec, tt * P:(tt + 1) * P]
                        for h in range(H):
                            nc.tensor.matmul(pks[h], ets, sb_wkT[h][:, ec, :],
                                             start=(ec == 0), stop=False)
                    for h in range(H):
                        nc.tensor.matmul(pks[h], convs[h][:, tt * P:(tt + 1) * P],
                                         sb_wattT[h], start=False, stop=False)
                    for h in range(H):
                        nc.tensor.matmul(pks[h], sb_ones_b, q_sb[h][b],
                                         start=False, stop=True)
                    for h in range(H):
                        th = tanhp.tile([P, DK], BF16, tag="tanh", name=f"tanh{b}_{ti}_{h}")
                        nc.scalar.activation(th, pks[h], mybir.ActivationFunctionType.Tanh)
                        scr = scrp.tile([P, DK], BF16, tag="scr", name=f"scr{b}_{ti}_{h}")
                        nc.vector.tensor_mul(scr, th, sb_gw[h])
                        nc.vector.reduce_sum(out=e_sb[:, ti, h:h + 1], in_=scr,
                                             axis=mybir.AxisListType.X)

        # ---- softmax + ws + context accumulation per batch ---------------------
        if stages < 3:
            return
        w_sbs = []
        for b in range(BL):
            e_sb = e_sbs[b]
            mk = softp.tile([P, NT, H], F32, tag="mask", name=f"mask{b}")
            nc.gpsimd.dma_start(out=mk, in_=ap["maskM"][b])
            nc.vector.tensor_add(e_sb, e_sb, mk)
            wexp = softp.tile([P, NT, H], F32, tag="wexp", name=f"wexp{b}")
            nc.scalar.activation(wexp, e_sb, mybir.ActivationFunctionType.Exp,
                                 scale=SCALING)
            part = softp.tile([P, H], F32, tag="part", name=f"part{b}")
            for h in range(H):
                nc.vector.reduce_sum(out=part[:, h:h + 1], in_=wexp[:, :, h],
                                     axis=mybir.AxisListType.X)
            pd = psum.tile([P, H], F32, tag="ps", name=f"pd{b}")
            nc.tensor.matmul(pd, sb_ones_f, part, start=True, stop=True)
            rbc = softp.tile([P, H], F32, tag="rbc", name=f"rbc{b}")
            nc.vector.reciprocal(rbc, pd)
            w_sb = softp.tile([P, NT, H], F32, tag="w", name=f"w{b}")
            for h in range(H):
                nc.vector.tensor_scalar_mul(w_sb[:, :, h], wexp[:, :, h], rbc[:, h:h + 1])
            w_sbs.append(w_sb)

            # ---- ws output: transpose [128t, 16tile] -> [16, 128] per head -----
            for h in range(H):
                pt = psum.tile([NT, P], F32, tag="ps", name=f"pt{b}_{h}")
                nc.tensor.matmul(pt, w_sb[:, :, h], sb_I, start=True, stop=True)
                wst = wstp.tile([NT, P], F32, tag="wst", name=f"wst{b}_{h}")
                nc.vector.tensor_copy(wst, pt)
                nc.gpsimd.dma_start(
                    out=ap["ws_out"][h, b, :].rearrange("(c p) -> c p", p=P),
                    in_=wst)

            # ---- phase 3: m[h, e] = sum_t w[t, h] * enc[t, e] ------------------
            if stages < 4:
                continue
            pms = [psum.tile([H, 512], F32, tag="ps", name=f"pm{b}_{eh}")
                   for eh in range(2)]
            for ti in range(NT):
                en = encNp.tile([P, E], F32, tag="encN", name=f"encN{b}_{ti}")
                nc.sync.dma_start(out=en[:, 0:512],
                                  in_=ap["encN"][b, ti * P:(ti + 1) * P, 0:512])
                nc.sync.dma_start(out=en[:, 512:E],
                                  in_=ap["encN"][b, ti * P:(ti + 1) * P, 512:E])
                for eh in range(2):
                    nc.tensor.matmul(pms[eh], w_sb[:, ti, :],
                                     en[:, eh * 512:(eh + 1) * 512],
                                     start=(ti == 0), stop=(ti == NT - 1))
            for eh in range(2):
                msb = mp.tile([H, 512], F32, tag="m", name=f"m{b}_{eh}")
                nc.vector.tensor_copy(msb, pms[eh])
                for cpart in range(4):
                    ec = eh * 4 + cpart
                    pmt = psum.tile([P, H], F32, tag="ps", name=f"pmt{b}_{ec}")
                    nc.tensor.matmul(pmt, msb[:, cpart * P:(cpart + 1) * P],
                                     sb_I[0:H, 0:H], start=True, stop=True)
                    nc.vector.tensor_copy(m2[:, ec, :, b], pmt)

        # ---- phase 4: c_h[v] = sum_e WvT[e, v] * m[e] (both batches at once) ---
        if stages < 5:
            return
        for h in range(H):
            wv_t = wvp.tile([P, NE, DV], F32, tag="wv", name=f"wv{h}")
            wv_v = ap["wvT"][h].rearrange("(c p) k -> p c k", p=P)
            nc.sync.dma_start(out=wv_t[:, 0:4, :], in_=wv_v[:, 0:4, :])
            nc.sync.dma_start(out=wv_t[:, 4:NE, :], in_=wv_v[:, 4:NE, :])
            pcs = [psum.tile([P, BL], F32, tag="ps", name=f"pc4_{h}_{vc}")
                   for vc in range(4)]
            for ec in range(NE):
                for vc in range(4):
                    nc.tensor.matmul(pcs[vc], wv_t[:, ec, vc * P:(vc + 1) * P],
                                     m2[:, ec, h, :],
                                     start=(ec == 0), stop=(ec == NE - 1))
            for vc in range(4):
                nc.vector.tensor_copy(cc[:, h * 4 + vc, :], pcs[vc])

        # ---- phase 5: c[b, :] = cc[:, b] @ WoT ---------------------------------
        woT_r = ap["woT"].rearrange("(c p) k -> p c k", p=P)   # [128, 16, 1024]
        for ih in range(2):
            po = psum.tile([BL, 512], F32, tag="ps", name=f"po{ih}")
            for half in range(2):
                wo_t = wop.tile([P, 8, 512], F32, tag="wo", name=f"wo{ih}_{half}")
                nc.sync.dma_start(
                    out=wo_t,
                    in_=woT_r[:, half * 8:(half + 1) * 8, ih * 512:(ih + 1) * 512])
                for j in range(8):
                    mc = half * 8 + j
                    nc.tensor.matmul(po, cc[:, mc, :], wo_t[:, j, :],
                                     start=(mc == 0), stop=(mc == HD // P - 1))
            csb = cp.tile([BL, 512], F32, tag="c", name=f"c{ih}")
            nc.vector.tensor_copy(csb, po)
            nc.gpsimd.dma_start(out=ap["c_out"][:, ih * 512:(ih + 1) * 512], in_=csb)


_INPUT_SPECS = [
    ("encT", [BL, E, T], BF16),
    ("encN", [BL, T, E], F32),
    ("xpad", [BL, TP], BF16),
    ("maskM", [BL, P, NT, H], F32),
    ("wkT", [H, E, DK], BF16),
    ("wattT", [H, C, DK], BF16),
    ("cw0", [W[0], C], BF16),
    ("cw1", [W[1], C], BF16),
    ("cw2", [W[2], C], BF16),
    ("cw3", [W[3], C], BF16),
    ("wqT", [H, DUP, DK], BF16),
    ("dzT", [DUP, BL], BF16),
    ("gw", [H, DK], BF16),
    ("wvT", [H, E, DV], F32),
    ("woT", [HD, E], F32),
    ("ident", [P, P], F32),
]
_OUTPUT_SPECS = [
    ("c_out", [BL, E], F32),
    ("ws_out", [H, BL, T], F32),
]


def build_program(stages=99):
    nc = bacc.Bacc("TRN2", target_bir_lowering=False, debug=False,
                   num_devices=NCORES)
    ap = {}
    for name, shape, dt in _INPUT_SPECS:
        ap[name] = nc.dram_tensor(name, shape, dt, kind="ExternalInput").ap()
    for name, shape, dt in _OUTPUT_SPECS:
        ap[name] = nc.dram_tensor(name, shape, dt, kind="ExternalOutput").ap()
    with tile.TileContext(nc) as tc:
        _emit(tc, ap, stages=stages)
    nc.compile()
    return nc


def make_in_maps(enc_hs_pad, enc_hs_len, dec_z, Wq, bq, Wk, Wv, g_w, g_b, Watt,
                 conv_w_0, conv_w_1, conv_w_2, conv_w_3, Wo):
    """Host-side sharding + layout prep. Returns list of per-core input dicts."""
    bf = ml_dtypes.bfloat16
    enc = np.asarray(enc_hs_pad, np.float32)
    lens = np.asarray(enc_hs_len).astype(np.int64)
    dec_z = np.asarray(dec_z, np.float32)
    Wq, Wk, Wv = (np.asarray(x, np.float32) for x in (Wq, Wk, Wv))
    bq, g_w, g_b = (np.asarray(x, np.float32) for x in (bq, g_w, g_b))
    Watt = np.asarray(Watt, np.float32)
    convs = [np.asarray(x, np.float32) for x in (conv_w_0, conv_w_1, conv_w_2, conv_w_3)]
    Wo = np.asarray(Wo, np.float32)

    # shared (all-core) weights; bq folded into wqT as row DU (dzT row DU = 1)
    wqT_aug = np.zeros((H, DUP, DK), np.float32)
    wqT_aug[:, :DU, :] = Wq.transpose(0, 2, 1)
    wqT_aug[:, DU, :] = bq
    shared = {
        "wkT": np.ascontiguousarray(Wk.transpose(0, 2, 1)).astype(bf),
        "wattT": np.ascontiguousarray(Watt.transpose(0, 2, 1)).astype(bf),
        "wqT": wqT_aug.astype(bf),
        "gw": np.ascontiguousarray(g_w).astype(bf),
        "wvT": np.ascontiguousarray(Wv.transpose(0, 2, 1)).astype(np.float32),
        "woT": np.ascontiguousarray(Wo.T).astype(np.float32),
        "ident": np.eye(P, dtype=np.float32),
    }
    for h in range(H):
        shared[f"cw{h}"] = np.ascontiguousarray(convs[h][:, 0, :].T).astype(bf)

    # att_prev (uniform over valid frames) and additive mask, from lengths
    pos = np.arange(T)[None, :]
    pad = pos >= lens[:, None]                                   # (B, T)
    x = np.where(pad, 0.0, 1.0 / lens[:, None].astype(np.float64)).astype(np.float32)

    in_maps = []
    for i in range(NCORES):
        b0 = i * BL
        encl = enc[b0:b0 + BL]
        xpad = np.zeros((BL, TP), np.float32)
        xpad[:, XOFF:XOFF + T] = x[b0:b0 + BL]
        # maskM[b, p, tile, h] = g_b[h] + (pad ? NEG : 0), t = tile*128 + p
        padl = pad[b0:b0 + BL].reshape(BL, NT, P).transpose(0, 2, 1)  # (BL, P, NT)
        maskM = (g_b[None, None, None, :]
                 + np.where(padl, NEG, 0.0)[:, :, :, None]).astype(np.float32)
        m = dict(shared)
        m["encT"] = np.ascontiguousarray(encl.transpose(0, 2, 1)).astype(bf)
        m["encN"] = np.ascontiguousarray(encl)
        m["xpad"] = xpad.astype(bf)
        m["maskM"] = np.ascontiguousarray(maskM)
        dzT_aug = np.zeros((DUP, BL), np.float32)
        dzT_aug[:DU, :] = dec_z[b0:b0 + BL].T
        dzT_aug[DU, :] = 1.0
        m["dzT"] = dzT_aug.astype(bf)
        in_maps.append(m)
    return in_maps


_NC_CACHE = None


def kernel(**inputs):
    """Full-input, full-output entry point. Returns (c, ws) like the reference."""
    global _NC_CACHE
    from concourse.bass_utils import run_bass_kernel_spmd

    in_maps = make_in_maps(**inputs)
    if _NC_CACHE is None:
        _NC_CACHE = build_program()
    res = run_bass_kernel_spmd(_NC_CACHE, in_maps, list(range(NCORES))).results

    c = np.empty((B, E), np.float32)
    ws = np.empty((H, B, T), np.float32)
    for i in range(NCORES):
        b0 = i * BL
        c[b0:b0 + BL] = res[i]["c_out"]
        ws[:, b0:b0 + BL, :] = res[i]["ws_out"]
    return c, ws


# revision 33
# speedup vs baseline: 1.0014x; 1.0014x over previous
"""Trainium2 Bass kernel: multi-head multi-resolution location-aware attention.

Math (per batch b, head h):
  K      = enc @ Wk[h].T                      (T, DK)   <- heavy matmul, bf16 on PE
  conv_h = depthwise 1-D conv of att_prev     (C, T)
  att    = conv_h.T @ Watt[h]                 (T, DK)
  Q      = Wq[h] @ dec_z + bq                 (DK,)
  e[t]   = g_w[h] . tanh(K[t] + att[t] + Q) + g_b[h]
  w      = softmax(scale * mask(e))           (T,)
  c_h    = Wv[h] @ (enc.T @ w)                <- V-projection eliminated algebraically
  c      = concat_h(c_h) @ Wo.T

Sharding: data-parallel over batch B=16 across 8 cores (2 per core), no collectives.
Each PSUM K-tile is [128t, 512k]; accumulation = 8 e-chunks + att_feat + Q (ones-row),
with all 4 heads sharing each stationary encT slice across parallel PSUM banks.
tanh on ScalarE (bf16 out), g*tanh -> sum_k as DVE mul+reduce (bf16, 2x mode),
softmax on-chip (no max-subtraction needed: |logits| < 1), context path in f32:
m = enc.T @ w on PE with w stationary, then Wv, then Wo. The im2col/encT loads are
issued two chunks ahead of their matmuls; big weight DMAs are split in halves so
transfers ride two HWDGE rings in parallel.
"""

import numpy as np
import ml_dtypes
from contextlib import ExitStack

import concourse.bass as bass
import concourse.bacc as bacc
import concourse.mybir as mybir
import concourse.tile as tile

F32 = mybir.dt.float32
BF16 = mybir.dt.bfloat16

# problem constants (hardcoded per contract)
B, T, E, DU = 16, 2048, 1024, 1024
H, DK, DV, C = 4, 512, 512, 128
HD = H * DV                      # 2048
NCORES = 8
BL = B // NCORES                 # 2 batches per core
P = 128
AF = [25, 50, 75, 100]           # per-head conv half-widths
W = [2 * a + 1 for a in AF]      # 51, 101, 151, 201
# per-head (offset, count) chunks of the conv-tap dimension
CHUNKS = [[(0, w)] if w <= P else [(0, P), (P, w - P)] for w in W]
XOFF = 128                       # left zero-pad of the conv input
TP = T + 2 * XOFF                # padded conv-input length
SCALING = 1.0 / float(np.sqrt(DK))
NEG = -1.0e30

NT = T // P                      # 16 t-tiles of 128
NTC = T // 512                   # 4 t-chunks of 512
NE = E // P                      # 8 e-chunks of 128
DUP = DU + P                     # dec_z padded with a constant-1 row (folds bq in)
NDC = DUP // P                   # 9 contraction chunks for the Q projection


def _emit(tc, ap, stages=99):
    nc = tc.nc
    ctx = ExitStack()

    sing = ctx.enter_context(tc.tile_pool(name="sing", bufs=1))
    psum = ctx.enter_context(tc.tile_pool(name="psum", bufs=8, space="PSUM"))
    wqp = ctx.enter_context(tc.tile_pool(name="wqp", bufs=2))
    encTp = ctx.enter_context(tc.tile_pool(name="encTp", bufs=3))
    xtp = ctx.enter_context(tc.tile_pool(name="xtp", bufs=12))
    convp = ctx.enter_context(tc.tile_pool(name="convp", bufs=8))
    tanhp = ctx.enter_context(tc.tile_pool(name="tanhp", bufs=5))
    scrp = ctx.enter_context(tc.tile_pool(name="scrp", bufs=3))
    softp = ctx.enter_context(tc.tile_pool(name="softp", bufs=2))
    wstp = ctx.enter_context(tc.tile_pool(name="wstp", bufs=2))
    encNp = ctx.enter_context(tc.tile_pool(name="encNp", bufs=2))
    mp = ctx.enter_context(tc.tile_pool(name="mp", bufs=2))
    wvp = ctx.enter_context(tc.tile_pool(name="wvp", bufs=1))
    wop = ctx.enter_context(tc.tile_pool(name="wop", bufs=2))
    cp = ctx.enter_context(tc.tile_pool(name="cp", bufs=2))

    with ctx:
        # ---- persistent weights -------------------------------------------------
        sb_wkT = []
        sb_wattT = []
        sb_gw = []
        sb_cw = []
        for h in range(H):
            wk = sing.tile([P, NE, DK], BF16, tag=f"wkT{h}", name=f"wkT{h}")
            nc.sync.dma_start(out=wk, in_=ap["wkT"][h].rearrange("(c p) k -> p c k", p=P))
            sb_wkT.append(wk)
            wa = sing.tile([C, DK], BF16, tag=f"wattT{h}", name=f"wattT{h}")
            nc.sync.dma_start(out=wa, in_=ap["wattT"][h])
            sb_wattT.append(wa)
            g_row = ap["gw"][h:h + 1, :]
            g_bc = bass.AP(tensor=g_row.tensor, offset=g_row.offset, ap=[[0, P], g_row.ap[1]])
            gt = sing.tile([P, DK], BF16, tag=f"gw{h}", name=f"gw{h}")
            nc.sync.dma_start(out=gt, in_=g_bc)
            sb_gw.append(gt)
            cws = []
            for j, (dofs, dcnt) in enumerate(CHUNKS[h]):
                cw = sing.tile([dcnt, C], BF16, tag=f"cw{h}_{j}", name=f"cw{h}_{j}")
                nc.sync.dma_start(out=cw, in_=ap[f"cw{h}"][dofs:dofs + dcnt, :])
                cws.append(cw)
            sb_cw.append(cws)

        sb_I = sing.tile([P, P], F32, tag="ident", name="identt")
        nc.sync.dma_start(out=sb_I, in_=ap["ident"])
        sb_dzT = sing.tile([P, NDC, BL], BF16, tag="dzT", name="dzTt")
        nc.sync.dma_start(out=sb_dzT, in_=ap["dzT"].rearrange("(c p) b -> p c b", p=P))
        sb_ones_b = sing.tile([1, P], BF16, tag="ones_b", name="ones_b")
        nc.vector.memset(sb_ones_b, 1.0)
        sb_ones_f = sing.tile([P, P], F32, tag="ones_f", name="ones_f")
        nc.vector.memset(sb_ones_f, 1.0)

        q_sb = [[sing.tile([1, DK], BF16, tag=f"q{h}_{b}", name=f"q{h}_{b}")
                 for b in range(BL)] for h in range(H)]
        m2 = sing.tile([P, NE, H, BL], F32, tag="m2", name="m2_sb")
        cc = sing.tile([P, HD // P, BL], F32, tag="cc", name="cc_sb")

        # ---- Q projection: q[h][b] = Wq[h] @ dec_z[b] + bq[h] (bq folded in) ---
        for h in range(H):
            wq_t = wqp.tile([P, NDC, DK], BF16, tag="wq", name=f"wq{h}")
            wq_v = ap["wqT"][h].rearrange("(c p) k -> p c k", p=P)
            nc.sync.dma_start(out=wq_t[:, 0:5, :], in_=wq_v[:, 0:5, :])
            nc.sync.dma_start(out=wq_t[:, 5:NDC, :], in_=wq_v[:, 5:NDC, :])
            pqs = [psum.tile([1, DK], F32, tag="ps", name=f"pq{h}_{b}")
                   for b in range(BL)]
            for dc in range(NDC):
                for b in range(BL):
                    nc.tensor.matmul(pqs[b], sb_dzT[:, dc, b:b + 1], wq_t[:, dc, :],
                                     start=(dc == 0), stop=(dc == NDC - 1))
            for b in range(BL):
                nc.vector.tensor_copy(q_sb[h][b], pqs[b])

        # ---- main loop: K-groups for BOTH batches first (keeps PE dense; the
        # ---- softmax/context phases below overlap with the other batch) --------
        if stages < 2:
            return
        def emit_dmas(b, tcc):
            """Issue the encT + im2col DMAs for (b, tcc) — two chunks ahead of
            the K-groups so transfers land long before the PE needs them."""
            encT_b = ap["encT"][b].rearrange("(c p) t -> p c t", p=P)
            et = encTp.tile([P, NE, 512], BF16, tag="encT", name=f"encT{b}_{tcc}")
            nc.sync.dma_start(out=et[:, 0:4, :],
                              in_=encT_b[:, 0:4, tcc * 512:(tcc + 1) * 512])
            nc.sync.dma_start(out=et[:, 4:NE, :],
                              in_=encT_b[:, 4:NE, tcc * 512:(tcc + 1) * 512])
            xts = []
            for h in range(H):
                for j, (dofs, dcnt) in enumerate(CHUNKS[h]):
                    xt = xtp.tile([P, 512], BF16, tag="xt", name=f"xt{b}_{tcc}_{h}_{j}")
                    base = b * TP + XOFF - AF[h] + tcc * 512 + dofs
                    src = bass.AP(tensor=ap["xpad"].tensor, offset=base,
                                  ap=[[1, dcnt], [1, 512]])
                    nc.sync.dma_start(out=xt[:dcnt, :], in_=src)
                    xts.append(xt)
            return et, xts

        def emit_conv_mms(b, tcc, xts):
            """Conv matmuls + PSUM->SBUF copies; emitted after the previous
            chunk's K-groups (PE reaches them with the DMA data already in)."""
            convs = []
            k = 0
            for h in range(H):
                pc = psum.tile([C, 512], F32, tag="ps", name=f"pconv{b}_{tcc}_{h}")
                nch = len(CHUNKS[h])
                for j, (dofs, dcnt) in enumerate(CHUNKS[h]):
                    nc.tensor.matmul(pc, sb_cw[h][j], xts[k][:dcnt, :],
                                     start=(j == 0), stop=(j == nch - 1))
                    k += 1
                cv = convp.tile([C, 512], BF16, tag="conv", name=f"conv{b}_{tcc}_{h}")
                nc.vector.tensor_copy(cv, pc)
                convs.append(cv)
            return convs

        e_sbs = [softp.tile([P, NT, H], F32, tag="e", name=f"e{b}")
                 for b in range(BL)]
        bt = [(b, tcc) for b in range(BL) for tcc in range(NTC)]
        dmas = {0: emit_dmas(*bt[0])}
        convd = {0: emit_conv_mms(*bt[0], dmas[0][1])}
        dmas[1] = emit_dmas(*bt[1])
        for idx, (b, tcc) in enumerate(bt):
            e_sb = e_sbs[b]
            et, _ = dmas.pop(idx)
            convs = convd.pop(idx)
            if idx + 2 < len(bt):
                dmas[idx + 2] = emit_dmas(*bt[idx + 2])
            if idx + 1 < len(bt):
                convd[idx + 1] = emit_conv_mms(*bt[idx + 1], dmas[idx + 1][1])
            if True:
                for tt in range(4):
                    ti = tcc * 4 + tt
                    # all 4 heads accumulate in parallel PSUM banks so each
                    # stationary encT slice is loaded once and reused 4x
                    pks = [psum.tile([P, DK], F32, tag="ps", name=f"pk{b}_{ti}_{h}")
                           for h in range(H)]
                    for ec in range(NE):
                        ets = et[:, ec, tt * P:(tt + 1) * P]
                        for h in range(H):
                            nc.tensor.matmul(pks[h], ets, sb_wkT[h][:, ec, :],
                                             start=(ec == 0), stop=False)
                    for h in range(H):
                        nc.tensor.matmul(pks[h], convs[h][:, tt * P:(tt + 1) * P],
                                         sb_wattT[h], start=False, stop=False)
                    for h in range(H):
                        nc.tensor.matmul(pks[h], sb_ones_b, q_sb[h][b],
                                         start=False, stop=True)
                    for h in range(H):
                        th = tanhp.tile([P, DK], BF16, tag="tanh", name=f"tanh{b}_{ti}_{h}")
                        nc.scalar.activation(th, pks[h], mybir.ActivationFunctionType.Tanh)
                        scr = scrp.tile([P, DK], BF16, tag="scr", name=f"scr{b}_{ti}_{h}")
                        nc.vector.tensor_mul(scr, th, sb_gw[h])
                        nc.vector.reduce_sum(out=e_sb[:, ti, h:h + 1], in_=scr,
                                             axis=mybir.AxisListType.X)

        # ---- softmax + ws + context accumulation per batch ---------------------
        if stages < 3:
            return
        w_sbs = []
        for b in range(BL):
            e_sb = e_sbs[b]
            mk = softp.tile([P, NT, H], F32, tag="mask", name=f"mask{b}")
            nc.gpsimd.dma_start(out=mk, in_=ap["maskM"][b])
            nc.vector.tensor_add(e_sb, e_sb, mk)
            wexp = softp.tile([P, NT, H], F32, tag="wexp", name=f"wexp{b}")
            nc.scalar.activation(wexp, e_sb, mybir.ActivationFunctionType.Exp,
                                 scale=SCALING)
            part = softp.tile([P, H], F32, tag="part", name=f"part{b}")
            for h in range(H):
                nc.vector.reduce_sum(out=part[:, h:h + 1], in_=wexp[:, :, h],
                                     axis=mybir.AxisListType.X)
            pd = psum.tile([P, H], F32, tag="ps", name=f"pd{b}")
            nc.tensor.matmul(pd, sb_ones_f, part, start=True, stop=True)
            rbc = softp.tile([P, H], F32, tag="rbc", name=f"rbc{b}")
            nc.vector.reciprocal(rbc, pd)
            w_sb = softp.tile([P, NT, H], F32, tag="w", name=f"w{b}")
            for h in range(H):
                nc.vector.tensor_scalar_mul(w_sb[:, :, h], wexp[:, :, h], rbc[:, h:h + 1])
            w_sbs.append(w_sb)

            # ---- ws output: transpose [128t, 16tile] -> [16, 128] per head -----
            for h in range(H):
                pt = psum.tile([NT, P], F32, tag="ps", name=f"pt{b}_{h}")
                nc.tensor.matmul(pt, w_sb[:, :, h], sb_I, start=True, stop=True)
                wst = wstp.tile([NT, P], F32, tag="wst", name=f"wst{b}_{h}")
                nc.vector.tensor_copy(wst, pt)
                nc.gpsimd.dma_start(
                    out=ap["ws_out"][h, b, :].rearrange("(c p) -> c p", p=P),
                    in_=wst)

            # ---- phase 3: m[h, e] = sum_t w[t, h] * enc[t, e] ------------------
            if stages < 4:
                continue
            pms = [psum.tile([H, 512], F32, tag="ps", name=f"pm{b}_{eh}")
                   for eh in range(2)]
            for ti in range(NT):
                en = encNp.tile([P, E], F32, tag="encN", name=f"encN{b}_{ti}")
                nc.sync.dma_start(out=en[:, 0:512],
                                  in_=ap["encN"][b, ti * P:(ti + 1) * P, 0:512])
                nc.sync.dma_start(out=en[:, 512:E],
                                  in_=ap["encN"][b, ti * P:(ti + 1) * P, 512:E])
                for eh in range(2):
                    nc.tensor.matmul(pms[eh], w_sb[:, ti, :],
                                     en[:, eh * 512:(eh + 1) * 512],
                                     start=(ti == 0), stop=(ti == NT - 1))
            for eh in range(2):
                msb = mp.tile([H, 512], F32, tag="m", name=f"m{b}_{eh}")
                nc.vector.tensor_copy(msb, pms[eh])
                for cpart in range(4):
                    ec = eh * 4 + cpart
                    pmt = psum.tile([P, H], F32, tag="ps", name=f"pmt{b}_{ec}")
                    nc.tensor.matmul(pmt, msb[:, cpart * P:(cpart + 1) * P],
                                     sb_I[0:H, 0:H], start=True, stop=True)
                    nc.vector.tensor_copy(m2[:, ec, :, b], pmt)

        # ---- phase 4: c_h[v] = sum_e WvT[e, v] * m[e] (both batches at once) ---
        if stages < 5:
            return
        for h in range(H):
            wv_t = wvp.tile([P, NE, DV], F32, tag="wv", name=f"wv{h}")
            wv_v = ap["wvT"][h].rearrange("(c p) k -> p c k", p=P)
            nc.sync.dma_start(out=wv_t[:, 0:4, :], in_=wv_v[:, 0:4, :])
            nc.sync.dma_start(out=wv_t[:, 4:NE, :], in_=wv_v[:, 4:NE, :])
            pcs = [psum.tile([P, BL], F32, tag="ps", name=f"pc4_{h}_{vc}")
                   for vc in range(4)]
            for ec in range(NE):
                for vc in range(4):
                    nc.tensor.matmul(pcs[vc], wv_t[:, ec, vc * P:(vc + 1) * P],
                                     m2[:, ec, h, :],
                                     start=(ec == 0), stop=(ec == NE - 1))
            for vc in range(4):
                nc.vector.tensor_copy(cc[:, h * 4 + vc, :], pcs[vc])

        # ---- phase 5: c[b, :] = cc[:, b] @ WoT ---------------------------------
        woT_r = ap["woT"].rearrange("(c p) k -> p c k", p=P)   # [128, 16, 1024]
        for ih in range(2):
            po = psum.tile([BL, 512], F32, tag="ps", name=f"po{ih}")
            for half in range(2):
                wo_t = wop.tile([P, 8, 512], F32, tag="wo", name=f"wo{ih}_{half}")
                nc.sync.dma_start(
                    out=wo_t,
                    in_=woT_r[:, half * 8:(half + 1) * 8, ih * 512:(ih + 1) * 512])
                for j in range(8):
                    mc = half * 8 + j
                    nc.tensor.matmul(po, cc[:, mc, :], wo_t[:, j, :],
                                     start=(mc == 0), stop=(mc == HD // P - 1))
            csb = cp.tile([BL, 512], F32, tag="c", name=f"c{ih}")
            nc.vector.tensor_copy(csb, po)
            nc.gpsimd.dma_start(out=ap["c_out"][:, ih * 512:(ih + 1) * 512], in_=csb)


_INPUT_SPECS = [
    ("encT", [BL, E, T], BF16),
    ("encN", [BL, T, E], F32),
    ("xpad", [BL, TP], BF16),
    ("maskM", [BL, P, NT, H], F32),
    ("wkT", [H, E, DK], BF16),
    ("wattT", [H, C, DK], BF16),
    ("cw0", [W[0], C], BF16),
    ("cw1", [W[1], C], BF16),
    ("cw2", [W[2], C], BF16),
    ("cw3", [W[3], C], BF16),
    ("wqT", [H, DUP, DK], BF16),
    ("dzT", [DUP, BL], BF16),
    ("gw", [H, DK], BF16),
    ("wvT", [H, E, DV], F32),
    ("woT", [HD, E], F32),
    ("ident", [P, P], F32),
]
_OUTPUT_SPECS = [
    ("c_out", [BL, E], F32),
    ("ws_out", [H, BL, T], F32),
]


def build_program(stages=99):
    nc = bacc.Bacc("TRN2", target_bir_lowering=False, debug=False,
                   num_devices=NCORES)
    ap = {}
    for name, shape, dt in _INPUT_SPECS:
        ap[name] = nc.dram_tensor(name, shape, dt, kind="ExternalInput").ap()
    for name, shape, dt in _OUTPUT_SPECS:
        ap[name] = nc.dram_tensor(name, shape, dt, kind="ExternalOutput").ap()
    with tile.TileContext(nc) as tc:
        _emit(tc, ap, stages=stages)
    nc.compile()
    return nc


def make_in_maps(enc_hs_pad, enc_hs_len, dec_z, Wq, bq, Wk, Wv, g_w, g_b, Watt,
                 conv_w_0, conv_w_1, conv_w_2, conv_w_3, Wo):
    """Host-side sharding + layout prep. Returns list of per-core input dicts."""
    bf = ml_dtypes.bfloat16
    enc = np.asarray(enc_hs_pad, np.float32)
    lens = np.asarray(enc_hs_len).astype(np.int64)
    dec_z = np.asarray(dec_z, np.float32)
    Wq, Wk, Wv = (np.asarray(x, np.float32) for x in (Wq, Wk, Wv))
    bq, g_w, g_b = (np.asarray(x, np.float32) for x in (bq, g_w, g_b))
    Watt = np.asarray(Watt, np.float32)
    convs = [np.asarray(x, np.float32) for x in (conv_w_0, conv_w_1, conv_w_2, conv_w_3)]
    Wo = np.asarray(Wo, np.float32)

    # shared (all-core) weights; bq folded into wqT as row DU (dzT row DU = 1)
    wqT_aug = np.zeros((H, DUP, DK), np.float32)
    wqT_aug[:, :DU, :] = Wq.transpose(0, 2, 1)
    wqT_aug[:, DU, :] = bq
    shared = {
        "wkT": np.ascontiguousarray(Wk.transpose(0, 2, 1)).astype(bf),
        "wattT": np.ascontiguousarray(Watt.transpose(0, 2, 1)).astype(bf),
        "wqT": wqT_aug.astype(bf),
        "gw": np.ascontiguousarray(g_w).astype(bf),
        "wvT": np.ascontiguousarray(Wv.transpose(0, 2, 1)).astype(np.float32),
        "woT": np.ascontiguousarray(Wo.T).astype(np.float32),
        "ident": np.eye(P, dtype=np.float32),
    }
    for h in range(H):
        shared[f"cw{h}"] = np.ascontiguousarray(convs[h][:, 0, :].T).astype(bf)

    # att_prev (uniform over valid frames) and additive mask, from lengths
    pos = np.arange(T)[None, :]
    pad = pos >= lens[:, None]                                   # (B, T)
    x = np.where(pad, 0.0, 1.0 / lens[:, None].astype(np.float64)).astype(np.float32)

    in_maps = []
    for i in range(NCORES):
        b0 = i * BL
        encl = enc[b0:b0 + BL]
        xpad = np.zeros((BL, TP), np.float32)
        xpad[:, XOFF:XOFF + T] = x[b0:b0 + BL]
        # maskM[b, p, tile, h] = g_b[h] + (pad ? NEG : 0), t = tile*128 + p
        padl = pad[b0:b0 + BL].reshape(BL, NT, P).transpose(0, 2, 1)  # (BL, P, NT)
        maskM = (g_b[None, None, None, :]
                 + np.where(padl, NEG, 0.0)[:, :, :, None]).astype(np.float32)
        m = dict(shared)
        m["encT"] = np.ascontiguousarray(encl.transpose(0, 2, 1)).astype(bf)
        m["encN"] = np.ascontiguousarray(encl)
        m["xpad"] = xpad.astype(bf)
        m["maskM"] = np.ascontiguousarray(maskM)
        dzT_aug = np.zeros((DUP, BL), np.float32)
        dzT_aug[:DU, :] = dec_z[b0:b0 + BL].T
        dzT_aug[DU, :] = 1.0
        m["dzT"] = dzT_aug.astype(bf)
        in_maps.append(m)
    return in_maps


_NC_CACHE = None


def kernel(**inputs):
    """Full-input, full-output entry point. Returns (c, ws) like the reference."""
    global _NC_CACHE
    from concourse.bass_utils import run_bass_kernel_spmd

    in_maps = make_in_maps(**inputs)
    if _NC_CACHE is None:
        _NC_CACHE = build_program()
    res = run_bass_kernel_spmd(_NC_CACHE, in_maps, list(range(NCORES))).results

    c = np.empty((B, E), np.float32)
    ws = np.empty((H, B, T), np.float32)
    for i in range(NCORES):
        b0 = i * BL
        c[b0:b0 + BL] = res[i]["c_out"]
        ws[:, b0:b0 + BL, :] = res[i]["ws_out"]
    return c, ws
